# revision 49
# baseline (speedup 1.0000x reference)
"""Trainium2 Bass kernel for nn_BitfieldLinear (vq_codebook).

Reference computation:
    idx   = codes & 0xFF            (basis row, 256 entries)
    r_q   = (codes >> 8) & 0xFFF
    sign  = bit20 ? -1 : +1
    scale = sign * tanh(r_q / 4095)
    W     = scale[:, None] * basis[idx]        # [8192, 4096]
    y     = x @ W.T                            # [128, 8192]

Key factorization (never materialize the 128MB W):
    Z = x @ basis.T                            # [128, 256]  tiny matmul
    y[b, j] = scale[j] * Z[b, idx[j]]          # column gather + scale

On-chip we compute only the UNSCALED gather (the per-output scale and
the code decode are pure functions of `codes`, applied on the host):
    G[k, j] = (idx[j] == k)                    # 0/1 one-hot  [256, 1024]
    y_core  = Z @ G                            # [128, 1024], host multiplies scale

Sharding: out_features column-parallel across 8 cores (1024 codes per
core); x and basis replicated.  Per core:
    1. stream x^T / basis^T interleaved per K-tile (fp16) across FOUR
       DMA rings: scalar + sync (HWDGE) and two gpsimd rings (direct +
       indirect-with-identity-offsets on qPoolDynamic).  The SWDGE
       rings use ~4x bigger packets, so the SDMA packet-round-robin
       gives them ~4x the bandwidth share: bytes are split
       6/6/10/10 bundles so all rings finish together at the ~320 GB/s
       aggregate the mixed-packet arbitration sustains.  The PE
       consumes bundles in the same proportional interleave and
       accumulates Z [128, 256] in PSUM.
    2. one-hot G built during the stream: PE rank-1 broadcast of the
       host-decoded idx row (ones[1,128].T @ idx[1,1024] -> PSUM), then
       one DVE is_equal per 128-row half against an iota column
    3. Z -> fp16, PE-transpose into Z^T, y = Z^T.T @ G via 8 fp16
       matmuls grouped by k-half (one LDWEIGHTS per half, 4 PSUM banks
       in flight), cast fp16, store in 4 chunks spread over three
       DMA queues
Host reassembles: y_full[:, c*1024+j] = scale[c*1024+j] * out_c[:, j].
"""

import sys

for _p in ("/opt/trn_rl_repo", "/opt/pypackages"):
    if _p not in sys.path:
        sys.path.insert(0, _p)

import numpy as np

import concourse.bacc as bacc
import concourse.mybir as mybir
import concourse.tile as tile
from concourse.alu_op_type import AluOpType
from concourse.bass_utils import run_bass_kernel_spmd

N_CORES = 8
BATCH = 128
IN_F = 4096
OUT_F = 8192
BASIS = 256
OPC = OUT_F // N_CORES      # 1024 output columns per core
NK = IN_F // 128            # 32 K-tiles
R_LEVELS = 4095.0

F32 = mybir.dt.float32
FP16 = mybir.dt.float16
I32 = mybir.dt.int32

BW = 128 + 256              # bundle cols: xt tile (128) + bt tile (256)
# ring bundle counts: scalar / sync (HWDGE, small packets) get 6 each;
# the two gpsimd rings (SWDGE, big packets -> bigger arbitration share)
# get 10 each
RING_BUNDLES = [1, 1, 18, 12]
RING_CHUNKS = [
    [1],
    [1],
    [6, 6, 6],
    [6, 6],
]
RING_OFF = np.cumsum([0] + RING_BUNDLES).tolist()
assert all(sum(c) == b for c, b in zip(RING_CHUNKS, RING_BUNDLES))

# PE consumption order: HWDGE head chunks first (they land earliest),
# then a proportional largest-remainder interleave across all rings.
def _pe_order():
    head = [(0, 0), (1, 0)]
    counts = [RING_BUNDLES[0] - 1, RING_BUNDLES[1] - 1,
              RING_BUNDLES[2], RING_BUNDLES[3]]
    offs = [1, 1, 0, 0]
    n = sum(counts)
    cons = [0] * 4
    order = list(head)
    for t in range(1, n + 1):
        r = max(range(4), key=lambda i: counts[i] * t / n - cons[i])
        order.append((r, offs[r] + cons[r]))
        cons[r] += 1
    return order

PE_ORDER = _pe_order()
assert sorted((r, k) for r, k in PE_ORDER) == [
    (r, k) for r in range(4) for k in range(RING_BUNDLES[r])
]


def build_nc():
    import concourse.bass as bass

    nc = bacc.Bacc(
        "TRN2",
        target_bir_lowering=False,
        debug=False,
        num_devices=N_CORES,
    )

    stream_ds = [
        [
            nc.dram_tensor(f"s{r}c{c}", [128, nb * BW], FP16,
                           kind="ExternalInput")
            for c, nb in enumerate(RING_CHUNKS[r])
        ]
        for r in range(len(RING_BUNDLES))
    ]
    idx_d = nc.dram_tensor("idx16", [1, OPC], FP16, kind="ExternalInput")
    ident_d = nc.dram_tensor("ident", [128, 128], FP16, kind="ExternalInput")
    out_d = nc.dram_tensor("out", [128, OPC], FP16, kind="ExternalOutput")

    with tile.TileContext(nc) as tc:
        with (
            tc.tile_pool(name="pool", bufs=1) as pool,
            tc.tile_pool(name="zps", bufs=1, space="PSUM") as zps,
            tc.tile_pool(name="bps", bufs=1, space="PSUM") as bps,
            tc.tile_pool(name="tps", bufs=1, space="PSUM") as tps,
            tc.tile_pool(name="yps", bufs=4, space="PSUM") as yps,
        ):
            idx_sb = pool.tile([1, OPC], FP16)
            ident = pool.tile([128, 128], FP16)
            ones = pool.tile([1, 128], FP16)
            nc.vector.memset(ones[:], 1.0)
            # identity row offsets for the indirect (qPool) stream ring,
            # generated on-chip so no DMA gates the first indirect issue
            ioff = pool.tile([128, 1], I32)
            nc.gpsimd.iota(ioff[:], [[0, 1]], channel_multiplier=1)
            # iota2[p, h] = p + 128h, the is_equal comparison columns —
            # generated on-chip (f32 exact for values < 256)
            iota2 = pool.tile([128, 2], F32)
            nc.gpsimd.iota(iota2[:], [[128, 2]], channel_multiplier=1,
                           allow_small_or_imprecise_dtypes=True)

            # ---- stream DMAs: ring r chunk c -> contiguous SBUF cols.
            # First stream chunks lead on every queue; tiny decode
            # inputs ride behind them on the HWDGE queues.
            stream_sb = pool.tile([128, NK * BW], FP16)
            engines = [nc.scalar, nc.sync, nc.gpsimd]
            for c in range(max(len(ch) for ch in RING_CHUNKS)):
                for r in range(len(RING_BUNDLES)):
                    if c >= len(RING_CHUNKS[r]):
                        continue
                    b0 = RING_OFF[r] + sum(RING_CHUNKS[r][:c])
                    nb = RING_CHUNKS[r][c]
                    dst = stream_sb[:, b0 * BW:(b0 + nb) * BW]
                    if r < 3:
                        engines[r].dma_start(out=dst, in_=stream_ds[r][c][:])
                    else:
                        nc.gpsimd.indirect_dma_start(
                            out=dst,
                            out_offset=None,
                            in_=stream_ds[r][c][:],
                            in_offset=bass.IndirectOffsetOnAxis(
                                ap=ioff[:, :1], axis=0,
                            ),
                        )
                if c == 0:
                    nc.scalar.dma_start(out=idx_sb[:], in_=idx_d[:])
                if c == 1:
                    # ident only needed for the Z transpose at the end;
                    # issued after sync's last stream chunk so it never
                    # contends with the stream ramp
                    nc.sync.dma_start(out=ident[:], in_=ident_d[:])

            # ---- Z accumulation, proportional round-robin across rings
            z_ps = zps.tile([128, BASIS], F32, tag="z")
            idx_bc = bps.tile([128, OPC], F32, tag="ib")
            g_sb = [
                pool.tile([128, OPC], FP16, tag=f"g{h}", name=f"g_sb{h}")
                for h in range(2)
            ]

            for s, (r, k) in enumerate(PE_ORDER):
                base = (RING_OFF[r] + k) * BW
                nc.tensor.matmul(
                    z_ps[:],
                    lhsT=stream_sb[:, base:base + 128],
                    rhs=stream_sb[:, base + 128:base + BW],
                    start=(s == 0), stop=(s == NK - 1),
                )
                if s == 2:
                    # rank-1 broadcast: idx_bc[p, j] = idx[j]
                    for half in range(2):
                        nc.tensor.matmul(
                            idx_bc[:, half * 512:(half + 1) * 512],
                            lhsT=ones[:],
                            rhs=idx_sb[:, half * 512:(half + 1) * 512],
                            start=True, stop=True,
                        )
                if s == 4:
                    # one-hot halves: g_h[p, j] = (idx[j] == p + 128h)
                    for h in range(2):
                        nc.vector.tensor_scalar(
                            out=g_sb[h][:], in0=idx_bc[:],
                            scalar1=iota2[:, h:h + 1], scalar2=None,
                            op0=AluOpType.is_equal,
                        )

            # ---- Z -> fp16 SBUF (one copy: single dependency hop),
            # PE-transpose into Z^T
            z_sb = pool.tile([128, BASIS], FP16)
            nc.vector.tensor_copy(out=z_sb[:], in_=z_ps[:])
            zt = [
                pool.tile([128, 128], FP16, tag=f"zt{h}", name=f"zt{h}")
                for h in range(2)
            ]
            tp = tps.tile([128, 256], FP16, tag="tp", name="ztp")
            for h in range(2):
                nc.tensor.transpose(
                    out=tp[:, h * 128:(h + 1) * 128],
                    in_=z_sb[:, h * 128:(h + 1) * 128],
                    identity=ident[:],
                )
            nc.vector.tensor_copy(out=zt[0][:], in_=tp[:, 0:128])
            nc.scalar.copy(out=zt[1][:], in_=tp[:, 128:256])

            # ---- y = Z^T.T @ G, four N-chunks of 256, grouped by
            # k-half so the stationary zt loads once per half; 4 PSUM
            # banks let all chunks stay in flight
            y_ps = [
                yps.tile([128, 256], F32, tag="y", name=f"y_ps{n}")
                for n in range(4)
            ]
            for h in range(2):
                for n in range(4):
                    nc.tensor.matmul(
                        y_ps[n][:],
                        lhsT=zt[h][:],
                        rhs=g_sb[h][:, n * 256:(n + 1) * 256],
                        start=(h == 0), stop=(h == 1),
                    )
            store_eng = [nc.sync, nc.scalar, nc.gpsimd, nc.sync]
            for n in range(4):
                y_sb = pool.tile([128, 256], FP16, tag=f"ysb{n}",
                                 name=f"y_sb{n}")
                if n % 2 == 0:
                    nc.vector.tensor_copy(out=y_sb[:], in_=y_ps[n][:])
                else:
                    nc.scalar.copy(out=y_sb[:], in_=y_ps[n][:])
                store_eng[n].dma_start(
                    out=out_d[:, n * 256:(n + 1) * 256], in_=y_sb[:]
                )

    nc.compile()
    return nc


_NC = None


def _get_nc():
    global _NC
    if _NC is None:
        _NC = build_nc()
    return _NC


def make_in_maps(x, codes, basis):
    x = np.ascontiguousarray(x, dtype=np.float32)
    basis = np.ascontiguousarray(basis, dtype=np.float32)
    codes = np.ascontiguousarray(codes, dtype=np.int32)

    # xt[p, n*128 + m] = x[m, n*128 + p]
    xt = (
        x.reshape(BATCH, NK, 128).transpose(2, 1, 0).astype(np.float16)
    )  # [128, NK, 128]
    # bt[p, n*256 + o] = basis[o, n*128 + p]
    bt = (
        basis.reshape(BASIS, NK, 128).transpose(2, 1, 0).astype(np.float16)
    )  # [128, NK, 256]
    # interleave per K-tile: bundle n = [xt_n | bt_n], then permute so
    # K-tile s sits at the SBUF slot PE_ORDER consumes at step s
    bundles = np.concatenate([xt, bt], axis=2)      # [128, NK, BW]
    perm = np.empty(NK, dtype=np.int64)
    for s, (r, k) in enumerate(PE_ORDER):
        perm[RING_OFF[r] + k] = s
    stream = bundles[:, perm, :].reshape(128, NK * BW)

    common = {}
    for r in range(len(RING_BUNDLES)):
        for c, nb in enumerate(RING_CHUNKS[r]):
            b0 = RING_OFF[r] + sum(RING_CHUNKS[r][:c])
            common[f"s{r}c{c}"] = np.ascontiguousarray(
                stream[:, b0 * BW:(b0 + nb) * BW]
            )
    common["ident"] = np.eye(128, dtype=np.float16)

    idx = (codes & 255).astype(np.float16)
    in_maps = []
    for c in range(N_CORES):
        m = dict(common)
        m["idx16"] = np.ascontiguousarray(
            idx[c * OPC:(c + 1) * OPC].reshape(1, OPC)
        )
        in_maps.append(m)
    return in_maps


def _host_scale(codes):
    r = ((codes >> 8) & 4095).astype(np.float32) / np.float32(R_LEVELS)
    sign = np.where(((codes >> 20) & 1) == 1, -1.0, 1.0).astype(np.float32)
    return sign * np.tanh(r)


def assemble_output(results, codes):
    y = np.concatenate(
        [results[c]["out"].astype(np.float32) for c in range(N_CORES)], axis=1
    )
    return y * _host_scale(np.asarray(codes, dtype=np.int32))[None, :]


def kernel(x, codes, basis):
    nc = _get_nc()
    in_maps = make_in_maps(x, codes, basis)
    res = run_bass_kernel_spmd(nc, in_maps, list(range(N_CORES)))
    return assemble_output(res.results, codes)


if __name__ == "__main__":
    rng = np.random.default_rng(0)
    x = rng.standard_normal((BATCH, IN_F), dtype=np.float32)
    basis = (rng.standard_normal((BASIS, IN_F)) * 0.02).astype(np.float32)
    codes = rng.integers(0, 1 << 22, size=(OUT_F,), dtype=np.int32)
    y = kernel(x, codes, basis)

    idx = codes & 255
    scale = _host_scale(codes)
    W = scale[:, None] * basis[idx]
    y_ref = x @ W.T
    err = np.linalg.norm(y - y_ref) / np.linalg.norm(y_ref)
    print("rel err:", err)


# revision 51
# speedup vs baseline: 1.0568x; 1.0568x over previous
"""Trainium2 Bass kernel for nn_BitfieldLinear (vq_codebook).

Reference computation:
    idx   = codes & 0xFF            (basis row, 256 entries)
    r_q   = (codes >> 8) & 0xFFF
    sign  = bit20 ? -1 : +1
    scale = sign * tanh(r_q / 4095)
    W     = scale[:, None] * basis[idx]        # [8192, 4096]
    y     = x @ W.T                            # [128, 8192]

Key factorization (never materialize the 128MB W):
    Z = x @ basis.T                            # [128, 256]  tiny matmul
    y[b, j] = scale[j] * Z[b, idx[j]]          # column gather + scale

On-chip we compute only the UNSCALED gather (the per-output scale and
the code decode are pure functions of `codes`, applied on the host):
    G[k, j] = (idx[j] == k)                    # 0/1 one-hot  [256, 1024]
    y_core  = Z @ G                            # [128, 1024], host multiplies scale

Sharding: out_features column-parallel across 8 cores (1024 codes per
core); x and basis replicated.  Per core:
    1. stream x^T / basis^T interleaved per K-tile (fp16) across FOUR
       DMA rings: scalar + sync (HWDGE) and two gpsimd rings (direct +
       indirect-with-identity-offsets on qPoolDynamic).  The SWDGE
       rings use ~4x bigger packets, so the SDMA packet-round-robin
       gives them ~4x the bandwidth share: bytes are split
       6/6/10/10 bundles so all rings finish together at the ~320 GB/s
       aggregate the mixed-packet arbitration sustains.  The PE
       consumes bundles in the same proportional interleave and
       accumulates Z [128, 256] in PSUM.
    2. one-hot G built during the stream: PE rank-1 broadcast of the
       host-decoded idx row (ones[1,128].T @ idx[1,1024] -> PSUM), then
       one DVE is_equal per 128-row half against an iota column
    3. Z -> fp16, PE-transpose into Z^T, y = Z^T.T @ G via 8 fp16
       matmuls grouped by k-half (one LDWEIGHTS per half, 4 PSUM banks
       in flight), cast fp16, store in 4 chunks spread over three
       DMA queues
Host reassembles: y_full[:, c*1024+j] = scale[c*1024+j] * out_c[:, j].
"""

import sys

for _p in ("/opt/trn_rl_repo", "/opt/pypackages"):
    if _p not in sys.path:
        sys.path.insert(0, _p)

import numpy as np

import concourse.bacc as bacc
import concourse.mybir as mybir
import concourse.tile as tile
from concourse.alu_op_type import AluOpType
from concourse.bass_utils import run_bass_kernel_spmd

N_CORES = 8
BATCH = 128
IN_F = 4096
OUT_F = 8192
BASIS = 256
OPC = OUT_F // N_CORES      # 1024 output columns per core
NK = IN_F // 128            # 32 K-tiles
R_LEVELS = 4095.0

F32 = mybir.dt.float32
FP16 = mybir.dt.float16
I32 = mybir.dt.int32

BW = 128 + 256              # bundle cols: xt tile (128) + bt tile (256)
# ring bundle counts: scalar / sync (HWDGE, small packets) get 6 each;
# the two gpsimd rings (SWDGE, big packets -> bigger arbitration share)
# get 10 each
RING_BUNDLES = [1, 1, 18, 12]
RING_CHUNKS = [
    [1],
    [1],
    [4, 5, 5, 4],
    [4, 4, 4],
]
RING_OFF = np.cumsum([0] + RING_BUNDLES).tolist()
assert all(sum(c) == b for c, b in zip(RING_CHUNKS, RING_BUNDLES))

# PE consumption order: HWDGE head chunks first (they land earliest),
# then a proportional largest-remainder interleave across all rings.
def _pe_order():
    head = [(0, 0), (1, 0)]
    counts = [RING_BUNDLES[0] - 1, RING_BUNDLES[1] - 1,
              RING_BUNDLES[2], RING_BUNDLES[3]]
    offs = [1, 1, 0, 0]
    n = sum(counts)
    cons = [0] * 4
    order = list(head)
    for t in range(1, n + 1):
        r = max(range(4), key=lambda i: counts[i] * t / n - cons[i])
        order.append((r, offs[r] + cons[r]))
        cons[r] += 1
    return order

PE_ORDER = _pe_order()
assert sorted((r, k) for r, k in PE_ORDER) == [
    (r, k) for r in range(4) for k in range(RING_BUNDLES[r])
]


def build_nc():
    import concourse.bass as bass

    nc = bacc.Bacc(
        "TRN2",
        target_bir_lowering=False,
        debug=False,
        num_devices=N_CORES,
    )

    stream_ds = [
        [
            nc.dram_tensor(f"s{r}c{c}", [128, nb * BW], FP16,
                           kind="ExternalInput")
            for c, nb in enumerate(RING_CHUNKS[r])
        ]
        for r in range(len(RING_BUNDLES))
    ]
    idx_d = nc.dram_tensor("idx16", [1, OPC], FP16, kind="ExternalInput")
    ident_d = nc.dram_tensor("ident", [128, 128], FP16, kind="ExternalInput")
    out_d = nc.dram_tensor("out", [128, OPC], FP16, kind="ExternalOutput")

    with tile.TileContext(nc) as tc:
        with (
            tc.tile_pool(name="pool", bufs=1) as pool,
            tc.tile_pool(name="zps", bufs=1, space="PSUM") as zps,
            tc.tile_pool(name="bps", bufs=1, space="PSUM") as bps,
            tc.tile_pool(name="tps", bufs=1, space="PSUM") as tps,
            tc.tile_pool(name="yps", bufs=4, space="PSUM") as yps,
        ):
            idx_sb = pool.tile([1, OPC], FP16)
            ident = pool.tile([128, 128], FP16)
            ones = pool.tile([1, 128], FP16)
            nc.vector.memset(ones[:], 1.0)
            # identity row offsets for the indirect (qPool) stream ring,
            # generated on-chip so no DMA gates the first indirect issue
            ioff = pool.tile([128, 1], I32)
            nc.gpsimd.iota(ioff[:], [[0, 1]], channel_multiplier=1)
            # iota2[p, h] = p + 128h, the is_equal comparison columns —
            # generated on-chip (f32 exact for values < 256)
            iota2 = pool.tile([128, 2], F32)
            nc.gpsimd.iota(iota2[:], [[128, 2]], channel_multiplier=1,
                           allow_small_or_imprecise_dtypes=True)

            # ---- stream DMAs: ring r chunk c -> contiguous SBUF cols.
            # First stream chunks lead on every queue; tiny decode
            # inputs ride behind them on the HWDGE queues.
            stream_sb = pool.tile([128, NK * BW], FP16)
            engines = [nc.scalar, nc.sync, nc.gpsimd]

            def emit_chunk(r, c):
                b0 = RING_OFF[r] + sum(RING_CHUNKS[r][:c])
                nb = RING_CHUNKS[r][c]
                dst = stream_sb[:, b0 * BW:(b0 + nb) * BW]
                if r < 3:
                    engines[r].dma_start(out=dst, in_=stream_ds[r][c][:])
                else:
                    nc.gpsimd.indirect_dma_start(
                        out=dst,
                        out_offset=None,
                        in_=stream_ds[r][c][:],
                        in_offset=bass.IndirectOffsetOnAxis(
                            ap=ioff[:, :1], axis=0,
                        ),
                    )

            # HWDGE head bundles + tiny inputs on their own queues;
            # gpsimd issues all DIRECT chunks first (the direct ring
            # saturates immediately), then the indirect chunks
            emit_chunk(0, 0)
            emit_chunk(1, 0)
            nc.scalar.dma_start(out=idx_sb[:], in_=idx_d[:])
            # ident only needed for the Z transpose at the end
            nc.sync.dma_start(out=ident[:], in_=ident_d[:])
            for c in range(len(RING_CHUNKS[2])):
                emit_chunk(2, c)
            for c in range(len(RING_CHUNKS[3])):
                emit_chunk(3, c)

            # ---- Z accumulation, proportional round-robin across rings
            z_ps = zps.tile([128, BASIS], F32, tag="z")
            idx_bc = bps.tile([128, OPC], F32, tag="ib")
            g_sb = [
                pool.tile([128, OPC], FP16, tag=f"g{h}", name=f"g_sb{h}")
                for h in range(2)
            ]

            for s, (r, k) in enumerate(PE_ORDER):
                base = (RING_OFF[r] + k) * BW
                nc.tensor.matmul(
                    z_ps[:],
                    lhsT=stream_sb[:, base:base + 128],
                    rhs=stream_sb[:, base + 128:base + BW],
                    start=(s == 0), stop=(s == NK - 1),
                )
                if s == 2:
                    # rank-1 broadcast: idx_bc[p, j] = idx[j]
                    for half in range(2):
                        nc.tensor.matmul(
                            idx_bc[:, half * 512:(half + 1) * 512],
                            lhsT=ones[:],
                            rhs=idx_sb[:, half * 512:(half + 1) * 512],
                            start=True, stop=True,
                        )
                if s == 4:
                    # one-hot halves: g_h[p, j] = (idx[j] == p + 128h)
                    for h in range(2):
                        nc.vector.tensor_scalar(
                            out=g_sb[h][:], in0=idx_bc[:],
                            scalar1=iota2[:, h:h + 1], scalar2=None,
                            op0=AluOpType.is_equal,
                        )

            # ---- Z -> fp16 SBUF (one copy: single dependency hop),
            # PE-transpose into Z^T
            z_sb = pool.tile([128, BASIS], FP16)
            nc.vector.tensor_copy(out=z_sb[:], in_=z_ps[:])
            zt = [
                pool.tile([128, 128], FP16, tag=f"zt{h}", name=f"zt{h}")
                for h in range(2)
            ]
            tp = tps.tile([128, 256], FP16, tag="tp", name="ztp")
            for h in range(2):
                nc.tensor.transpose(
                    out=tp[:, h * 128:(h + 1) * 128],
                    in_=z_sb[:, h * 128:(h + 1) * 128],
                    identity=ident[:],
                )
            nc.vector.tensor_copy(out=zt[0][:], in_=tp[:, 0:128])
            nc.scalar.copy(out=zt[1][:], in_=tp[:, 128:256])

            # ---- y = Z^T.T @ G, four N-chunks of 256, grouped by
            # k-half so the stationary zt loads once per half; 4 PSUM
            # banks let all chunks stay in flight
            y_ps = [
                yps.tile([128, 256], F32, tag="y", name=f"y_ps{n}")
                for n in range(4)
            ]
            for h in range(2):
                for n in range(4):
                    nc.tensor.matmul(
                        y_ps[n][:],
                        lhsT=zt[h][:],
                        rhs=g_sb[h][:, n * 256:(n + 1) * 256],
                        start=(h == 0), stop=(h == 1),
                    )
            store_eng = [nc.sync, nc.scalar, nc.gpsimd, nc.sync]
            for n in range(4):
                y_sb = pool.tile([128, 256], FP16, tag=f"ysb{n}",
                                 name=f"y_sb{n}")
                if n % 2 == 0:
                    nc.vector.tensor_copy(out=y_sb[:], in_=y_ps[n][:])
                else:
                    nc.scalar.copy(out=y_sb[:], in_=y_ps[n][:])
                store_eng[n].dma_start(
                    out=out_d[:, n * 256:(n + 1) * 256], in_=y_sb[:]
                )

    nc.compile()
    return nc


_NC = None


def _get_nc():
    global _NC
    if _NC is None:
        _NC = build_nc()
    return _NC


def make_in_maps(x, codes, basis):
    x = np.ascontiguousarray(x, dtype=np.float32)
    basis = np.ascontiguousarray(basis, dtype=np.float32)
    codes = np.ascontiguousarray(codes, dtype=np.int32)

    # xt[p, n*128 + m] = x[m, n*128 + p]
    xt = (
        x.reshape(BATCH, NK, 128).transpose(2, 1, 0).astype(np.float16)
    )  # [128, NK, 128]
    # bt[p, n*256 + o] = basis[o, n*128 + p]
    bt = (
        basis.reshape(BASIS, NK, 128).transpose(2, 1, 0).astype(np.float16)
    )  # [128, NK, 256]
    # interleave per K-tile: bundle n = [xt_n | bt_n], then permute so
    # K-tile s sits at the SBUF slot PE_ORDER consumes at step s
    bundles = np.concatenate([xt, bt], axis=2)      # [128, NK, BW]
    perm = np.empty(NK, dtype=np.int64)
    for s, (r, k) in enumerate(PE_ORDER):
        perm[RING_OFF[r] + k] = s
    stream = bundles[:, perm, :].reshape(128, NK * BW)

    common = {}
    for r in range(len(RING_BUNDLES)):
        for c, nb in enumerate(RING_CHUNKS[r]):
            b0 = RING_OFF[r] + sum(RING_CHUNKS[r][:c])
            common[f"s{r}c{c}"] = np.ascontiguousarray(
                stream[:, b0 * BW:(b0 + nb) * BW]
            )
    common["ident"] = np.eye(128, dtype=np.float16)

    idx = (codes & 255).astype(np.float16)
    in_maps = []
    for c in range(N_CORES):
        m = dict(common)
        m["idx16"] = np.ascontiguousarray(
            idx[c * OPC:(c + 1) * OPC].reshape(1, OPC)
        )
        in_maps.append(m)
    return in_maps


def _host_scale(codes):
    r = ((codes >> 8) & 4095).astype(np.float32) / np.float32(R_LEVELS)
    sign = np.where(((codes >> 20) & 1) == 1, -1.0, 1.0).astype(np.float32)
    return sign * np.tanh(r)


def assemble_output(results, codes):
    y = np.concatenate(
        [results[c]["out"].astype(np.float32) for c in range(N_CORES)], axis=1
    )
    return y * _host_scale(np.asarray(codes, dtype=np.int32))[None, :]


def kernel(x, codes, basis):
    nc = _get_nc()
    in_maps = make_in_maps(x, codes, basis)
    res = run_bass_kernel_spmd(nc, in_maps, list(range(N_CORES)))
    return assemble_output(res.results, codes)


if __name__ == "__main__":
    rng = np.random.default_rng(0)
    x = rng.standard_normal((BATCH, IN_F), dtype=np.float32)
    basis = (rng.standard_normal((BASIS, IN_F)) * 0.02).astype(np.float32)
    codes = rng.integers(0, 1 << 22, size=(OUT_F,), dtype=np.int32)
    y = kernel(x, codes, basis)

    idx = codes & 255
    scale = _host_scale(codes)
    W = scale[:, None] * basis[idx]
    y_ref = x @ W.T
    err = np.linalg.norm(y - y_ref) / np.linalg.norm(y_ref)
    print("rel err:", err)


# revision 52
# speedup vs baseline: 1.0894x; 1.0308x over previous
"""Trainium2 Bass kernel for nn_BitfieldLinear (vq_codebook).

Reference computation:
    idx   = codes & 0xFF            (basis row, 256 entries)
    r_q   = (codes >> 8) & 0xFFF
    sign  = bit20 ? -1 : +1
    scale = sign * tanh(r_q / 4095)
    W     = scale[:, None] * basis[idx]        # [8192, 4096]
    y     = x @ W.T                            # [128, 8192]

Key factorization (never materialize the 128MB W):
    Z = x @ basis.T                            # [128, 256]  tiny matmul
    y[b, j] = scale[j] * Z[b, idx[j]]          # column gather + scale

On-chip we compute only the UNSCALED gather (the per-output scale and
the code decode are pure functions of `codes`, applied on the host):
    G[k, j] = (idx[j] == k)                    # 0/1 one-hot  [256, 1024]
    y_core  = Z @ G                            # [128, 1024], host multiplies scale

Sharding: out_features column-parallel across 8 cores (1024 codes per
core); x and basis replicated.  Per core:
    1. stream x^T / basis^T interleaved per K-tile (fp16) across FOUR
       DMA rings: scalar + sync (HWDGE) and two gpsimd rings (direct +
       indirect-with-identity-offsets on qPoolDynamic).  The SWDGE
       rings use ~4x bigger packets, so the SDMA packet-round-robin
       gives them ~4x the bandwidth share: bytes are split
       6/6/10/10 bundles so all rings finish together at the ~320 GB/s
       aggregate the mixed-packet arbitration sustains.  The PE
       consumes bundles in the same proportional interleave and
       accumulates Z [128, 256] in PSUM.
    2. one-hot G built during the stream: PE rank-1 broadcast of the
       host-decoded idx row (ones[1,128].T @ idx[1,1024] -> PSUM), then
       one DVE is_equal per 128-row half against an iota column
    3. Z -> fp16, PE-transpose into Z^T, y = Z^T.T @ G via 8 fp16
       matmuls grouped by k-half (one LDWEIGHTS per half, 4 PSUM banks
       in flight), cast fp16, store in 4 chunks spread over three
       DMA queues
Host reassembles: y_full[:, c*1024+j] = scale[c*1024+j] * out_c[:, j].
"""

import sys

for _p in ("/opt/trn_rl_repo", "/opt/pypackages"):
    if _p not in sys.path:
        sys.path.insert(0, _p)

import numpy as np

import concourse.bacc as bacc
import concourse.mybir as mybir
import concourse.tile as tile
from concourse.alu_op_type import AluOpType
from concourse.bass_utils import run_bass_kernel_spmd

N_CORES = 8
BATCH = 128
IN_F = 4096
OUT_F = 8192
BASIS = 256
OPC = OUT_F // N_CORES      # 1024 output columns per core
NK = IN_F // 128            # 32 K-tiles
R_LEVELS = 4095.0

F32 = mybir.dt.float32
FP16 = mybir.dt.float16
I32 = mybir.dt.int32

BW = 128 + 256              # bundle cols: xt tile (128) + bt tile (256)
# ring bundle counts: scalar / sync (HWDGE, small packets) get 6 each;
# the two gpsimd rings (SWDGE, big packets -> bigger arbitration share)
# get 10 each
RING_BUNDLES = [1, 1, 18, 12]
RING_CHUNKS = [
    [1],
    [1],
    [4, 5, 5, 4],
    [4, 4, 4],
]
RING_OFF = np.cumsum([0] + RING_BUNDLES).tolist()
assert all(sum(c) == b for c, b in zip(RING_CHUNKS, RING_BUNDLES))

# PE consumption order: HWDGE head chunks first (they land earliest),
# then a proportional largest-remainder interleave across all rings.
def _pe_order():
    head = [(0, 0), (1, 0)]
    counts = [RING_BUNDLES[0] - 1, RING_BUNDLES[1] - 1,
              RING_BUNDLES[2], RING_BUNDLES[3]]
    offs = [1, 1, 0, 0]
    n = sum(counts)
    cons = [0] * 4
    order = list(head)
    for t in range(1, n + 1):
        r = max(range(4), key=lambda i: counts[i] * t / n - cons[i])
        order.append((r, offs[r] + cons[r]))
        cons[r] += 1
    return order

PE_ORDER = _pe_order()
assert sorted((r, k) for r, k in PE_ORDER) == [
    (r, k) for r in range(4) for k in range(RING_BUNDLES[r])
]


def build_nc():
    import concourse.bass as bass

    nc = bacc.Bacc(
        "TRN2",
        target_bir_lowering=False,
        debug=False,
        num_devices=N_CORES,
    )

    stream_ds = [
        [
            nc.dram_tensor(f"s{r}c{c}", [128, nb * BW], FP16,
                           kind="ExternalInput")
            for c, nb in enumerate(RING_CHUNKS[r])
        ]
        for r in range(len(RING_BUNDLES))
    ]
    idx_d = nc.dram_tensor("idx16", [1, OPC], FP16, kind="ExternalInput")
    ident_d = nc.dram_tensor("ident", [128, 128], FP16, kind="ExternalInput")
    out_d = nc.dram_tensor("out", [128, OPC], FP16, kind="ExternalOutput")

    with tile.TileContext(nc) as tc:
        with (
            tc.tile_pool(name="pool", bufs=1) as pool,
            tc.tile_pool(name="zps", bufs=1, space="PSUM") as zps,
            tc.tile_pool(name="bps", bufs=1, space="PSUM") as bps,
            tc.tile_pool(name="tps", bufs=1, space="PSUM") as tps,
            tc.tile_pool(name="yps", bufs=4, space="PSUM") as yps,
        ):
            idx_sb = pool.tile([1, OPC], FP16)
            ident = pool.tile([128, 128], FP16)
            ones = pool.tile([1, 128], FP16)
            nc.vector.memset(ones[:], 1.0)
            # identity row offsets for the indirect (qPool) stream ring,
            # generated on-chip so no DMA gates the first indirect issue
            ioff = pool.tile([128, 1], I32)
            nc.gpsimd.iota(ioff[:], [[0, 1]], channel_multiplier=1)
            # iota2[p, h] = p + 128h, the is_equal comparison columns —
            # generated on-chip (f32 exact for values < 256)
            iota2 = pool.tile([128, 2], F32)
            nc.gpsimd.iota(iota2[:], [[128, 2]], channel_multiplier=1,
                           allow_small_or_imprecise_dtypes=True)

            # ---- stream DMAs: ring r chunk c -> contiguous SBUF cols.
            # First stream chunks lead on every queue; tiny decode
            # inputs ride behind them on the HWDGE queues.
            stream_sb = pool.tile([128, NK * BW], FP16)
            engines = [nc.scalar, nc.sync, nc.gpsimd]

            def emit_chunk(r, c):
                b0 = RING_OFF[r] + sum(RING_CHUNKS[r][:c])
                nb = RING_CHUNKS[r][c]
                dst = stream_sb[:, b0 * BW:(b0 + nb) * BW]
                if r < 3:
                    engines[r].dma_start(out=dst, in_=stream_ds[r][c][:])
                else:
                    nc.gpsimd.indirect_dma_start(
                        out=dst,
                        out_offset=None,
                        in_=stream_ds[r][c][:],
                        in_offset=bass.IndirectOffsetOnAxis(
                            ap=ioff[:, :1], axis=0,
                        ),
                    )

            # First stream chunks lead on every queue; the gpsimd rings
            # interleave direct/indirect chunks so both rings ramp
            # together.  Tiny decode inputs ride behind the heads.
            for c in range(max(len(ch) for ch in RING_CHUNKS)):
                for r in range(len(RING_BUNDLES)):
                    if c < len(RING_CHUNKS[r]):
                        emit_chunk(r, c)
                if c == 0:
                    nc.scalar.dma_start(out=idx_sb[:], in_=idx_d[:])
                if c == 1:
                    # ident only needed for the Z transpose at the end
                    nc.sync.dma_start(out=ident[:], in_=ident_d[:])

            # ---- Z accumulation, proportional round-robin across rings
            z_ps = zps.tile([128, BASIS], F32, tag="z")
            idx_bc = bps.tile([128, OPC], F32, tag="ib")
            g_sb = [
                pool.tile([128, OPC], FP16, tag=f"g{h}", name=f"g_sb{h}")
                for h in range(2)
            ]

            for s, (r, k) in enumerate(PE_ORDER):
                base = (RING_OFF[r] + k) * BW
                nc.tensor.matmul(
                    z_ps[:],
                    lhsT=stream_sb[:, base:base + 128],
                    rhs=stream_sb[:, base + 128:base + BW],
                    start=(s == 0), stop=(s == NK - 1),
                )
                if s == 2:
                    # rank-1 broadcast: idx_bc[p, j] = idx[j]
                    for half in range(2):
                        nc.tensor.matmul(
                            idx_bc[:, half * 512:(half + 1) * 512],
                            lhsT=ones[:],
                            rhs=idx_sb[:, half * 512:(half + 1) * 512],
                            start=True, stop=True,
                        )
                if s == 4:
                    # one-hot halves: g_h[p, j] = (idx[j] == p + 128h)
                    for h in range(2):
                        nc.vector.tensor_scalar(
                            out=g_sb[h][:], in0=idx_bc[:],
                            scalar1=iota2[:, h:h + 1], scalar2=None,
                            op0=AluOpType.is_equal,
                        )

            # ---- Z -> fp16 SBUF (one copy: single dependency hop),
            # PE-transpose into Z^T
            z_sb = pool.tile([128, BASIS], FP16)
            nc.vector.tensor_copy(out=z_sb[:], in_=z_ps[:])
            zt = [
                pool.tile([128, 128], FP16, tag=f"zt{h}", name=f"zt{h}")
                for h in range(2)
            ]
            tp = tps.tile([128, 256], FP16, tag="tp", name="ztp")
            for h in range(2):
                nc.tensor.transpose(
                    out=tp[:, h * 128:(h + 1) * 128],
                    in_=z_sb[:, h * 128:(h + 1) * 128],
                    identity=ident[:],
                )
            nc.vector.tensor_copy(out=zt[0][:], in_=tp[:, 0:128])
            nc.scalar.copy(out=zt[1][:], in_=tp[:, 128:256])

            # ---- y = Z^T.T @ G, four N-chunks of 256, grouped by
            # k-half so the stationary zt loads once per half; 4 PSUM
            # banks let all chunks stay in flight
            y_ps = [
                yps.tile([128, 256], F32, tag="y", name=f"y_ps{n}")
                for n in range(4)
            ]
            for h in range(2):
                for n in range(4):
                    nc.tensor.matmul(
                        y_ps[n][:],
                        lhsT=zt[h][:],
                        rhs=g_sb[h][:, n * 256:(n + 1) * 256],
                        start=(h == 0), stop=(h == 1),
                    )
            store_eng = [nc.sync, nc.scalar, nc.gpsimd, nc.sync]
            for n in range(4):
                y_sb = pool.tile([128, 256], FP16, tag=f"ysb{n}",
                                 name=f"y_sb{n}")
                if n % 2 == 0:
                    nc.vector.tensor_copy(out=y_sb[:], in_=y_ps[n][:])
                else:
                    nc.scalar.copy(out=y_sb[:], in_=y_ps[n][:])
                store_eng[n].dma_start(
                    out=out_d[:, n * 256:(n + 1) * 256], in_=y_sb[:]
                )

    nc.compile()
    return nc


_NC = None


def _get_nc():
    global _NC
    if _NC is None:
        _NC = build_nc()
    return _NC


def make_in_maps(x, codes, basis):
    x = np.ascontiguousarray(x, dtype=np.float32)
    basis = np.ascontiguousarray(basis, dtype=np.float32)
    codes = np.ascontiguousarray(codes, dtype=np.int32)

    # xt[p, n*128 + m] = x[m, n*128 + p]
    xt = (
        x.reshape(BATCH, NK, 128).transpose(2, 1, 0).astype(np.float16)
    )  # [128, NK, 128]
    # bt[p, n*256 + o] = basis[o, n*128 + p]
    bt = (
        basis.reshape(BASIS, NK, 128).transpose(2, 1, 0).astype(np.float16)
    )  # [128, NK, 256]
    # interleave per K-tile: bundle n = [xt_n | bt_n], then permute so
    # K-tile s sits at the SBUF slot PE_ORDER consumes at step s
    bundles = np.concatenate([xt, bt], axis=2)      # [128, NK, BW]
    perm = np.empty(NK, dtype=np.int64)
    for s, (r, k) in enumerate(PE_ORDER):
        perm[RING_OFF[r] + k] = s
    stream = bundles[:, perm, :].reshape(128, NK * BW)

    common = {}
    for r in range(len(RING_BUNDLES)):
        for c, nb in enumerate(RING_CHUNKS[r]):
            b0 = RING_OFF[r] + sum(RING_CHUNKS[r][:c])
            common[f"s{r}c{c}"] = np.ascontiguousarray(
                stream[:, b0 * BW:(b0 + nb) * BW]
            )
    common["ident"] = np.eye(128, dtype=np.float16)

    idx = (codes & 255).astype(np.float16)
    in_maps = []
    for c in range(N_CORES):
        m = dict(common)
        m["idx16"] = np.ascontiguousarray(
            idx[c * OPC:(c + 1) * OPC].reshape(1, OPC)
        )
        in_maps.append(m)
    return in_maps


def _host_scale(codes):
    r = ((codes >> 8) & 4095).astype(np.float32) / np.float32(R_LEVELS)
    sign = np.where(((codes >> 20) & 1) == 1, -1.0, 1.0).astype(np.float32)
    return sign * np.tanh(r)


def assemble_output(results, codes):
    y = np.concatenate(
        [results[c]["out"].astype(np.float32) for c in range(N_CORES)], axis=1
    )
    return y * _host_scale(np.asarray(codes, dtype=np.int32))[None, :]


def kernel(x, codes, basis):
    nc = _get_nc()
    in_maps = make_in_maps(x, codes, basis)
    res = run_bass_kernel_spmd(nc, in_maps, list(range(N_CORES)))
    return assemble_output(res.results, codes)


if __name__ == "__main__":
    rng = np.random.default_rng(0)
    x = rng.standard_normal((BATCH, IN_F), dtype=np.float32)
    basis = (rng.standard_normal((BASIS, IN_F)) * 0.02).astype(np.float32)
    codes = rng.integers(0, 1 << 22, size=(OUT_F,), dtype=np.int32)
    y = kernel(x, codes, basis)

    idx = codes & 255
    scale = _host_scale(codes)
    W = scale[:, None] * basis[idx]
    y_ref = x @ W.T
    err = np.linalg.norm(y - y_ref) / np.linalg.norm(y_ref)
    print("rel err:", err)


# revision 53
# speedup vs baseline: 1.1163x; 1.0247x over previous
"""Trainium2 Bass kernel for nn_BitfieldLinear (vq_codebook).

Reference computation:
    idx   = codes & 0xFF            (basis row, 256 entries)
    r_q   = (codes >> 8) & 0xFFF
    sign  = bit20 ? -1 : +1
    scale = sign * tanh(r_q / 4095)
    W     = scale[:, None] * basis[idx]        # [8192, 4096]
    y     = x @ W.T                            # [128, 8192]

Key factorization (never materialize the 128MB W):
    Z = x @ basis.T                            # [128, 256]  tiny matmul
    y[b, j] = scale[j] * Z[b, idx[j]]          # column gather + scale

On-chip we compute only the UNSCALED gather (the per-output scale and
the code decode are pure functions of `codes`, applied on the host):
    G[k, j] = (idx[j] == k)                    # 0/1 one-hot  [256, 1024]
    y_core  = Z @ G                            # [128, 1024], host multiplies scale

Sharding: out_features column-parallel across 8 cores (1024 codes per
core); x and basis replicated.  Per core:
    1. stream x^T / basis^T interleaved per K-tile (fp16) across FOUR
       DMA rings: scalar + sync (HWDGE) and two gpsimd rings (direct +
       indirect-with-identity-offsets on qPoolDynamic).  The SWDGE
       rings use ~4x bigger packets, so the SDMA packet-round-robin
       gives them ~4x the bandwidth share: bytes are split
       6/6/10/10 bundles so all rings finish together at the ~320 GB/s
       aggregate the mixed-packet arbitration sustains.  The PE
       consumes bundles in the same proportional interleave and
       accumulates Z [128, 256] in PSUM.
    2. one-hot G built during the stream: PE rank-1 broadcast of the
       host-decoded idx row (ones[1,128].T @ idx[1,1024] -> PSUM), then
       one DVE is_equal per 128-row half against an iota column
    3. Z -> fp16, PE-transpose into Z^T, y = Z^T.T @ G via 8 fp16
       matmuls grouped by k-half (one LDWEIGHTS per half, 4 PSUM banks
       in flight), cast fp16, store in 4 chunks spread over three
       DMA queues
Host reassembles: y_full[:, c*1024+j] = scale[c*1024+j] * out_c[:, j].
"""

import sys

for _p in ("/opt/trn_rl_repo", "/opt/pypackages"):
    if _p not in sys.path:
        sys.path.insert(0, _p)

import numpy as np

import concourse.bacc as bacc
import concourse.mybir as mybir
import concourse.tile as tile
from concourse.alu_op_type import AluOpType
from concourse.bass_utils import run_bass_kernel_spmd

N_CORES = 8
BATCH = 128
IN_F = 4096
OUT_F = 8192
BASIS = 256
OPC = OUT_F // N_CORES      # 1024 output columns per core
NK = IN_F // 128            # 32 K-tiles
R_LEVELS = 4095.0

F32 = mybir.dt.float32
FP16 = mybir.dt.float16
I32 = mybir.dt.int32

BW = 128 + 256              # bundle cols: xt tile (128) + bt tile (256)
# ring bundle counts: scalar / sync (HWDGE, small packets) get 6 each;
# the two gpsimd rings (SWDGE, big packets -> bigger arbitration share)
# get 10 each
RING_BUNDLES = [2, 2, 16, 12]
RING_CHUNKS = [
    [2],
    [2],
    [4, 4, 4, 4],
    [4, 4, 4],
]
RING_OFF = np.cumsum([0] + RING_BUNDLES).tolist()
assert all(sum(c) == b for c, b in zip(RING_CHUNKS, RING_BUNDLES))

# PE consumption order: HWDGE head chunks first (they land earliest),
# then a proportional largest-remainder interleave across all rings.
def _pe_order():
    head = [(0, 0), (1, 0)]
    counts = [RING_BUNDLES[0] - 1, RING_BUNDLES[1] - 1,
              RING_BUNDLES[2], RING_BUNDLES[3]]
    offs = [1, 1, 0, 0]
    n = sum(counts)
    cons = [0] * 4
    order = list(head)
    for t in range(1, n + 1):
        r = max(range(4), key=lambda i: counts[i] * t / n - cons[i])
        order.append((r, offs[r] + cons[r]))
        cons[r] += 1
    return order

PE_ORDER = _pe_order()
assert sorted((r, k) for r, k in PE_ORDER) == [
    (r, k) for r in range(4) for k in range(RING_BUNDLES[r])
]


def build_nc():
    import concourse.bass as bass

    nc = bacc.Bacc(
        "TRN2",
        target_bir_lowering=False,
        debug=False,
        num_devices=N_CORES,
    )

    stream_ds = [
        [
            nc.dram_tensor(f"s{r}c{c}", [128, nb * BW], FP16,
                           kind="ExternalInput")
            for c, nb in enumerate(RING_CHUNKS[r])
        ]
        for r in range(len(RING_BUNDLES))
    ]
    idx_d = nc.dram_tensor("idx16", [1, OPC], FP16, kind="ExternalInput")
    ident_d = nc.dram_tensor("ident", [128, 128], FP16, kind="ExternalInput")
    out_d = nc.dram_tensor("out", [128, OPC], FP16, kind="ExternalOutput")

    with tile.TileContext(nc) as tc:
        with (
            tc.tile_pool(name="pool", bufs=1) as pool,
            tc.tile_pool(name="zps", bufs=1, space="PSUM") as zps,
            tc.tile_pool(name="bps", bufs=1, space="PSUM") as bps,
            tc.tile_pool(name="tps", bufs=1, space="PSUM") as tps,
            tc.tile_pool(name="yps", bufs=4, space="PSUM") as yps,
        ):
            idx_sb = pool.tile([1, OPC], FP16)
            ident = pool.tile([128, 128], FP16)
            ones = pool.tile([1, 128], FP16)
            nc.vector.memset(ones[:], 1.0)
            # identity row offsets for the indirect (qPool) stream ring,
            # generated on-chip so no DMA gates the first indirect issue
            ioff = pool.tile([128, 1], I32)
            nc.gpsimd.iota(ioff[:], [[0, 1]], channel_multiplier=1)
            # iota2[p, h] = p + 128h, the is_equal comparison columns —
            # generated on-chip (f32 exact for values < 256)
            iota2 = pool.tile([128, 2], F32)
            nc.gpsimd.iota(iota2[:], [[128, 2]], channel_multiplier=1,
                           allow_small_or_imprecise_dtypes=True)

            # ---- stream DMAs: ring r chunk c -> contiguous SBUF cols.
            # First stream chunks lead on every queue; tiny decode
            # inputs ride behind them on the HWDGE queues.
            stream_sb = pool.tile([128, NK * BW], FP16)
            engines = [nc.scalar, nc.sync, nc.gpsimd]

            def emit_chunk(r, c):
                b0 = RING_OFF[r] + sum(RING_CHUNKS[r][:c])
                nb = RING_CHUNKS[r][c]
                dst = stream_sb[:, b0 * BW:(b0 + nb) * BW]
                if r < 3:
                    engines[r].dma_start(out=dst, in_=stream_ds[r][c][:])
                else:
                    nc.gpsimd.indirect_dma_start(
                        out=dst,
                        out_offset=None,
                        in_=stream_ds[r][c][:],
                        in_offset=bass.IndirectOffsetOnAxis(
                            ap=ioff[:, :1], axis=0,
                        ),
                    )

            # First stream chunks lead on every queue; the gpsimd rings
            # interleave direct/indirect chunks so both rings ramp
            # together.  Tiny decode inputs ride behind the heads.
            for c in range(max(len(ch) for ch in RING_CHUNKS)):
                for r in range(len(RING_BUNDLES)):
                    if c < len(RING_CHUNKS[r]):
                        emit_chunk(r, c)
                if c == 0:
                    nc.scalar.dma_start(out=idx_sb[:], in_=idx_d[:])
                if c == 1:
                    # ident only needed for the Z transpose at the end
                    nc.sync.dma_start(out=ident[:], in_=ident_d[:])

            # ---- Z accumulation, proportional round-robin across rings
            z_ps = zps.tile([128, BASIS], F32, tag="z")
            idx_bc = bps.tile([128, OPC], F32, tag="ib")
            g_sb = [
                pool.tile([128, OPC], FP16, tag=f"g{h}", name=f"g_sb{h}")
                for h in range(2)
            ]

            for s, (r, k) in enumerate(PE_ORDER):
                base = (RING_OFF[r] + k) * BW
                nc.tensor.matmul(
                    z_ps[:],
                    lhsT=stream_sb[:, base:base + 128],
                    rhs=stream_sb[:, base + 128:base + BW],
                    start=(s == 0), stop=(s == NK - 1),
                )
                if s == 2:
                    # rank-1 broadcast: idx_bc[p, j] = idx[j]
                    for half in range(2):
                        nc.tensor.matmul(
                            idx_bc[:, half * 512:(half + 1) * 512],
                            lhsT=ones[:],
                            rhs=idx_sb[:, half * 512:(half + 1) * 512],
                            start=True, stop=True,
                        )
                if s == 4:
                    # one-hot halves: g_h[p, j] = (idx[j] == p + 128h)
                    for h in range(2):
                        nc.vector.tensor_scalar(
                            out=g_sb[h][:], in0=idx_bc[:],
                            scalar1=iota2[:, h:h + 1], scalar2=None,
                            op0=AluOpType.is_equal,
                        )

            # ---- Z -> fp16 SBUF (one copy: single dependency hop),
            # PE-transpose into Z^T
            z_sb = pool.tile([128, BASIS], FP16)
            nc.vector.tensor_copy(out=z_sb[:], in_=z_ps[:])
            zt = [
                pool.tile([128, 128], FP16, tag=f"zt{h}", name=f"zt{h}")
                for h in range(2)
            ]
            tp = tps.tile([128, 256], FP16, tag="tp", name="ztp")
            for h in range(2):
                nc.tensor.transpose(
                    out=tp[:, h * 128:(h + 1) * 128],
                    in_=z_sb[:, h * 128:(h + 1) * 128],
                    identity=ident[:],
                )
            nc.vector.tensor_copy(out=zt[0][:], in_=tp[:, 0:128])
            nc.scalar.copy(out=zt[1][:], in_=tp[:, 128:256])

            # ---- y = Z^T.T @ G, four N-chunks of 256, grouped by
            # k-half so the stationary zt loads once per half; 4 PSUM
            # banks let all chunks stay in flight
            y_ps = [
                yps.tile([128, 256], F32, tag="y", name=f"y_ps{n}")
                for n in range(4)
            ]
            for h in range(2):
                for n in range(4):
                    nc.tensor.matmul(
                        y_ps[n][:],
                        lhsT=zt[h][:],
                        rhs=g_sb[h][:, n * 256:(n + 1) * 256],
                        start=(h == 0), stop=(h == 1),
                    )
            store_eng = [nc.sync, nc.scalar, nc.gpsimd, nc.sync]
            for n in range(4):
                y_sb = pool.tile([128, 256], FP16, tag=f"ysb{n}",
                                 name=f"y_sb{n}")
                if n % 2 == 0:
                    nc.vector.tensor_copy(out=y_sb[:], in_=y_ps[n][:])
                else:
                    nc.scalar.copy(out=y_sb[:], in_=y_ps[n][:])
                store_eng[n].dma_start(
                    out=out_d[:, n * 256:(n + 1) * 256], in_=y_sb[:]
                )

    nc.compile()
    return nc


_NC = None


def _get_nc():
    global _NC
    if _NC is None:
        _NC = build_nc()
    return _NC


def make_in_maps(x, codes, basis):
    x = np.ascontiguousarray(x, dtype=np.float32)
    basis = np.ascontiguousarray(basis, dtype=np.float32)
    codes = np.ascontiguousarray(codes, dtype=np.int32)

    # xt[p, n*128 + m] = x[m, n*128 + p]
    xt = (
        x.reshape(BATCH, NK, 128).transpose(2, 1, 0).astype(np.float16)
    )  # [128, NK, 128]
    # bt[p, n*256 + o] = basis[o, n*128 + p]
    bt = (
        basis.reshape(BASIS, NK, 128).transpose(2, 1, 0).astype(np.float16)
    )  # [128, NK, 256]
    # interleave per K-tile: bundle n = [xt_n | bt_n], then permute so
    # K-tile s sits at the SBUF slot PE_ORDER consumes at step s
    bundles = np.concatenate([xt, bt], axis=2)      # [128, NK, BW]
    perm = np.empty(NK, dtype=np.int64)
    for s, (r, k) in enumerate(PE_ORDER):
        perm[RING_OFF[r] + k] = s
    stream = bundles[:, perm, :].reshape(128, NK * BW)

    common = {}
    for r in range(len(RING_BUNDLES)):
        for c, nb in enumerate(RING_CHUNKS[r]):
            b0 = RING_OFF[r] + sum(RING_CHUNKS[r][:c])
            common[f"s{r}c{c}"] = np.ascontiguousarray(
                stream[:, b0 * BW:(b0 + nb) * BW]
            )
    common["ident"] = np.eye(128, dtype=np.float16)

    idx = (codes & 255).astype(np.float16)
    in_maps = []
    for c in range(N_CORES):
        m = dict(common)
        m["idx16"] = np.ascontiguousarray(
            idx[c * OPC:(c + 1) * OPC].reshape(1, OPC)
        )
        in_maps.append(m)
    return in_maps


def _host_scale(codes):
    r = ((codes >> 8) & 4095).astype(np.float32) / np.float32(R_LEVELS)
    sign = np.where(((codes >> 20) & 1) == 1, -1.0, 1.0).astype(np.float32)
    return sign * np.tanh(r)


def assemble_output(results, codes):
    y = np.concatenate(
        [results[c]["out"].astype(np.float32) for c in range(N_CORES)], axis=1
    )
    return y * _host_scale(np.asarray(codes, dtype=np.int32))[None, :]


def kernel(x, codes, basis):
    nc = _get_nc()
    in_maps = make_in_maps(x, codes, basis)
    res = run_bass_kernel_spmd(nc, in_maps, list(range(N_CORES)))
    return assemble_output(res.results, codes)


if __name__ == "__main__":
    rng = np.random.default_rng(0)
    x = rng.standard_normal((BATCH, IN_F), dtype=np.float32)
    basis = (rng.standard_normal((BASIS, IN_F)) * 0.02).astype(np.float32)
    codes = rng.integers(0, 1 << 22, size=(OUT_F,), dtype=np.int32)
    y = kernel(x, codes, basis)

    idx = codes & 255
    scale = _host_scale(codes)
    W = scale[:, None] * basis[idx]
    y_ref = x @ W.T
    err = np.linalg.norm(y - y_ref) / np.linalg.norm(y_ref)
    print("rel err:", err)


# revision 54
# speedup vs baseline: 1.1210x; 1.0042x over previous
"""Trainium2 Bass kernel for nn_BitfieldLinear (vq_codebook).

Reference computation:
    idx   = codes & 0xFF            (basis row, 256 entries)
    r_q   = (codes >> 8) & 0xFFF
    sign  = bit20 ? -1 : +1
    scale = sign * tanh(r_q / 4095)
    W     = scale[:, None] * basis[idx]        # [8192, 4096]
    y     = x @ W.T                            # [128, 8192]

Key factorization (never materialize the 128MB W):
    Z = x @ basis.T                            # [128, 256]  tiny matmul
    y[b, j] = scale[j] * Z[b, idx[j]]          # column gather + scale

On-chip we compute only the UNSCALED gather (the per-output scale and
the code decode are pure functions of `codes`, applied on the host):
    G[k, j] = (idx[j] == k)                    # 0/1 one-hot  [256, 1024]
    y_core  = Z @ G                            # [128, 1024], host multiplies scale

Sharding: out_features column-parallel across 8 cores (1024 codes per
core); x and basis replicated.  Per core:
    1. stream x^T / basis^T interleaved per K-tile (fp16) across FOUR
       DMA rings: scalar + sync (HWDGE) and two gpsimd rings (direct +
       indirect-with-identity-offsets on qPoolDynamic).  The SWDGE
       rings use ~4x bigger packets, so the SDMA packet-round-robin
       gives them ~4x the bandwidth share: bytes are split
       6/6/10/10 bundles so all rings finish together at the ~320 GB/s
       aggregate the mixed-packet arbitration sustains.  The PE
       consumes bundles in the same proportional interleave and
       accumulates Z [128, 256] in PSUM.
    2. one-hot G built during the stream: PE rank-1 broadcast of the
       host-decoded idx row (ones[1,128].T @ idx[1,1024] -> PSUM), then
       one DVE is_equal per 128-row half against an iota column
    3. Z -> fp16, PE-transpose into Z^T, y = Z^T.T @ G via 8 fp16
       matmuls grouped by k-half (one LDWEIGHTS per half, 4 PSUM banks
       in flight), cast fp16, store in 4 chunks spread over three
       DMA queues
Host reassembles: y_full[:, c*1024+j] = scale[c*1024+j] * out_c[:, j].
"""

import sys

for _p in ("/opt/trn_rl_repo", "/opt/pypackages"):
    if _p not in sys.path:
        sys.path.insert(0, _p)

import numpy as np

import concourse.bacc as bacc
import concourse.mybir as mybir
import concourse.tile as tile
from concourse.alu_op_type import AluOpType
from concourse.bass_utils import run_bass_kernel_spmd

N_CORES = 8
BATCH = 128
IN_F = 4096
OUT_F = 8192
BASIS = 256
OPC = OUT_F // N_CORES      # 1024 output columns per core
NK = IN_F // 128            # 32 K-tiles
R_LEVELS = 4095.0

F32 = mybir.dt.float32
FP16 = mybir.dt.float16
I32 = mybir.dt.int32

BW = 128 + 256              # bundle cols: xt tile (128) + bt tile (256)
# ring bundle counts: scalar / sync (HWDGE, small packets) get 6 each;
# the two gpsimd rings (SWDGE, big packets -> bigger arbitration share)
# get 10 each
RING_BUNDLES = [3, 3, 15, 11]
RING_CHUNKS = [
    [3],
    [3],
    [4, 4, 4, 3],
    [4, 4, 3],
]
RING_OFF = np.cumsum([0] + RING_BUNDLES).tolist()
assert all(sum(c) == b for c, b in zip(RING_CHUNKS, RING_BUNDLES))

# PE consumption order: HWDGE head chunks first (they land earliest),
# then a proportional largest-remainder interleave across all rings.
def _pe_order():
    head = [(0, 0), (1, 0)]
    counts = [RING_BUNDLES[0] - 1, RING_BUNDLES[1] - 1,
              RING_BUNDLES[2], RING_BUNDLES[3]]
    offs = [1, 1, 0, 0]
    n = sum(counts)
    cons = [0] * 4
    order = list(head)
    for t in range(1, n + 1):
        r = max(range(4), key=lambda i: counts[i] * t / n - cons[i])
        order.append((r, offs[r] + cons[r]))
        cons[r] += 1
    return order

PE_ORDER = _pe_order()
assert sorted((r, k) for r, k in PE_ORDER) == [
    (r, k) for r in range(4) for k in range(RING_BUNDLES[r])
]


def build_nc():
    import concourse.bass as bass

    nc = bacc.Bacc(
        "TRN2",
        target_bir_lowering=False,
        debug=False,
        num_devices=N_CORES,
    )

    stream_ds = [
        [
            nc.dram_tensor(f"s{r}c{c}", [128, nb * BW], FP16,
                           kind="ExternalInput")
            for c, nb in enumerate(RING_CHUNKS[r])
        ]
        for r in range(len(RING_BUNDLES))
    ]
    idx_d = nc.dram_tensor("idx16", [1, OPC], FP16, kind="ExternalInput")
    ident_d = nc.dram_tensor("ident", [128, 128], FP16, kind="ExternalInput")
    out_d = nc.dram_tensor("out", [128, OPC], FP16, kind="ExternalOutput")

    with tile.TileContext(nc) as tc:
        with (
            tc.tile_pool(name="pool", bufs=1) as pool,
            tc.tile_pool(name="zps", bufs=1, space="PSUM") as zps,
            tc.tile_pool(name="bps", bufs=1, space="PSUM") as bps,
            tc.tile_pool(name="tps", bufs=1, space="PSUM") as tps,
            tc.tile_pool(name="yps", bufs=4, space="PSUM") as yps,
        ):
            idx_sb = pool.tile([1, OPC], FP16)
            ident = pool.tile([128, 128], FP16)
            ones = pool.tile([1, 128], FP16)
            nc.vector.memset(ones[:], 1.0)
            # identity row offsets for the indirect (qPool) stream ring,
            # generated on-chip so no DMA gates the first indirect issue
            ioff = pool.tile([128, 1], I32)
            nc.gpsimd.iota(ioff[:], [[0, 1]], channel_multiplier=1)
            # iota2[p, h] = p + 128h, the is_equal comparison columns —
            # generated on-chip (f32 exact for values < 256)
            iota2 = pool.tile([128, 2], F32)
            nc.gpsimd.iota(iota2[:], [[128, 2]], channel_multiplier=1,
                           allow_small_or_imprecise_dtypes=True)

            # ---- stream DMAs: ring r chunk c -> contiguous SBUF cols.
            # First stream chunks lead on every queue; tiny decode
            # inputs ride behind them on the HWDGE queues.
            stream_sb = pool.tile([128, NK * BW], FP16)
            engines = [nc.scalar, nc.sync, nc.gpsimd]

            def emit_chunk(r, c):
                b0 = RING_OFF[r] + sum(RING_CHUNKS[r][:c])
                nb = RING_CHUNKS[r][c]
                dst = stream_sb[:, b0 * BW:(b0 + nb) * BW]
                if r < 3:
                    engines[r].dma_start(out=dst, in_=stream_ds[r][c][:])
                else:
                    nc.gpsimd.indirect_dma_start(
                        out=dst,
                        out_offset=None,
                        in_=stream_ds[r][c][:],
                        in_offset=bass.IndirectOffsetOnAxis(
                            ap=ioff[:, :1], axis=0,
                        ),
                    )

            # First stream chunks lead on every queue; the gpsimd rings
            # interleave direct/indirect chunks so both rings ramp
            # together.  Tiny decode inputs ride behind the heads.
            for c in range(max(len(ch) for ch in RING_CHUNKS)):
                for r in range(len(RING_BUNDLES)):
                    if c < len(RING_CHUNKS[r]):
                        emit_chunk(r, c)
                if c == 0:
                    nc.scalar.dma_start(out=idx_sb[:], in_=idx_d[:])
                if c == 1:
                    # ident only needed for the Z transpose at the end
                    nc.sync.dma_start(out=ident[:], in_=ident_d[:])

            # ---- Z accumulation, proportional round-robin across rings
            z_ps = zps.tile([128, BASIS], F32, tag="z")
            idx_bc = bps.tile([128, OPC], F32, tag="ib")
            g_sb = [
                pool.tile([128, OPC], FP16, tag=f"g{h}", name=f"g_sb{h}")
                for h in range(2)
            ]

            for s, (r, k) in enumerate(PE_ORDER):
                base = (RING_OFF[r] + k) * BW
                nc.tensor.matmul(
                    z_ps[:],
                    lhsT=stream_sb[:, base:base + 128],
                    rhs=stream_sb[:, base + 128:base + BW],
                    start=(s == 0), stop=(s == NK - 1),
                )
                if s == 2:
                    # rank-1 broadcast: idx_bc[p, j] = idx[j]
                    for half in range(2):
                        nc.tensor.matmul(
                            idx_bc[:, half * 512:(half + 1) * 512],
                            lhsT=ones[:],
                            rhs=idx_sb[:, half * 512:(half + 1) * 512],
                            start=True, stop=True,
                        )
                if s == 4:
                    # one-hot halves: g_h[p, j] = (idx[j] == p + 128h)
                    for h in range(2):
                        nc.vector.tensor_scalar(
                            out=g_sb[h][:], in0=idx_bc[:],
                            scalar1=iota2[:, h:h + 1], scalar2=None,
                            op0=AluOpType.is_equal,
                        )

            # ---- Z -> fp16 SBUF (one copy: single dependency hop),
            # PE-transpose into Z^T
            z_sb = pool.tile([128, BASIS], FP16)
            nc.vector.tensor_copy(out=z_sb[:], in_=z_ps[:])
            zt = [
                pool.tile([128, 128], FP16, tag=f"zt{h}", name=f"zt{h}")
                for h in range(2)
            ]
            tp = tps.tile([128, 256], FP16, tag="tp", name="ztp")
            for h in range(2):
                nc.tensor.transpose(
                    out=tp[:, h * 128:(h + 1) * 128],
                    in_=z_sb[:, h * 128:(h + 1) * 128],
                    identity=ident[:],
                )
            nc.vector.tensor_copy(out=zt[0][:], in_=tp[:, 0:128])
            nc.scalar.copy(out=zt[1][:], in_=tp[:, 128:256])

            # ---- y = Z^T.T @ G, four N-chunks of 256, grouped by
            # k-half so the stationary zt loads once per half; 4 PSUM
            # banks let all chunks stay in flight
            y_ps = [
                yps.tile([128, 256], F32, tag="y", name=f"y_ps{n}")
                for n in range(4)
            ]
            for h in range(2):
                for n in range(4):
                    nc.tensor.matmul(
                        y_ps[n][:],
                        lhsT=zt[h][:],
                        rhs=g_sb[h][:, n * 256:(n + 1) * 256],
                        start=(h == 0), stop=(h == 1),
                    )
            store_eng = [nc.sync, nc.scalar, nc.gpsimd, nc.sync]
            for n in range(4):
                y_sb = pool.tile([128, 256], FP16, tag=f"ysb{n}",
                                 name=f"y_sb{n}")
                if n % 2 == 0:
                    nc.vector.tensor_copy(out=y_sb[:], in_=y_ps[n][:])
                else:
                    nc.scalar.copy(out=y_sb[:], in_=y_ps[n][:])
                store_eng[n].dma_start(
                    out=out_d[:, n * 256:(n + 1) * 256], in_=y_sb[:]
                )

    nc.compile()
    return nc


_NC = None


def _get_nc():
    global _NC
    if _NC is None:
        _NC = build_nc()
    return _NC


def make_in_maps(x, codes, basis):
    x = np.ascontiguousarray(x, dtype=np.float32)
    basis = np.ascontiguousarray(basis, dtype=np.float32)
    codes = np.ascontiguousarray(codes, dtype=np.int32)

    # xt[p, n*128 + m] = x[m, n*128 + p]
    xt = (
        x.reshape(BATCH, NK, 128).transpose(2, 1, 0).astype(np.float16)
    )  # [128, NK, 128]
    # bt[p, n*256 + o] = basis[o, n*128 + p]
    bt = (
        basis.reshape(BASIS, NK, 128).transpose(2, 1, 0).astype(np.float16)
    )  # [128, NK, 256]
    # interleave per K-tile: bundle n = [xt_n | bt_n], then permute so
    # K-tile s sits at the SBUF slot PE_ORDER consumes at step s
    bundles = np.concatenate([xt, bt], axis=2)      # [128, NK, BW]
    perm = np.empty(NK, dtype=np.int64)
    for s, (r, k) in enumerate(PE_ORDER):
        perm[RING_OFF[r] + k] = s
    stream = bundles[:, perm, :].reshape(128, NK * BW)

    common = {}
    for r in range(len(RING_BUNDLES)):
        for c, nb in enumerate(RING_CHUNKS[r]):
            b0 = RING_OFF[r] + sum(RING_CHUNKS[r][:c])
            common[f"s{r}c{c}"] = np.ascontiguousarray(
                stream[:, b0 * BW:(b0 + nb) * BW]
            )
    common["ident"] = np.eye(128, dtype=np.float16)

    idx = (codes & 255).astype(np.float16)
    in_maps = []
    for c in range(N_CORES):
        m = dict(common)
        m["idx16"] = np.ascontiguousarray(
            idx[c * OPC:(c + 1) * OPC].reshape(1, OPC)
        )
        in_maps.append(m)
    return in_maps


def _host_scale(codes):
    r = ((codes >> 8) & 4095).astype(np.float32) / np.float32(R_LEVELS)
    sign = np.where(((codes >> 20) & 1) == 1, -1.0, 1.0).astype(np.float32)
    return sign * np.tanh(r)


def assemble_output(results, codes):
    y = np.concatenate(
        [results[c]["out"].astype(np.float32) for c in range(N_CORES)], axis=1
    )
    return y * _host_scale(np.asarray(codes, dtype=np.int32))[None, :]


def kernel(x, codes, basis):
    nc = _get_nc()
    in_maps = make_in_maps(x, codes, basis)
    res = run_bass_kernel_spmd(nc, in_maps, list(range(N_CORES)))
    return assemble_output(res.results, codes)


if __name__ == "__main__":
    rng = np.random.default_rng(0)
    x = rng.standard_normal((BATCH, IN_F), dtype=np.float32)
    basis = (rng.standard_normal((BASIS, IN_F)) * 0.02).astype(np.float32)
    codes = rng.integers(0, 1 << 22, size=(OUT_F,), dtype=np.int32)
    y = kernel(x, codes, basis)

    idx = codes & 255
    scale = _host_scale(codes)
    W = scale[:, None] * basis[idx]
    y_ref = x @ W.T
    err = np.linalg.norm(y - y_ref) / np.linalg.norm(y_ref)
    print("rel err:", err)
